# revision 53
# baseline (speedup 1.0000x reference)
"""Trainium2 Bass kernel for nn_DynamicRouting (capsule dynamic routing).

Raw-bass implementation (manual engine programs + semaphores; the Tile
scheduler emits >1 sync-wait on DMA instructions, which this toolchain's
walrus rejects, so DMA waits are standalone engine instructions here).

Math (per batch element):
    for t in range(iters):
        c = softmax_j(b_vec);  s = einsum('ji,jin->jn', c, u) + bias
        v = squash(s)
        if t < iters-1: b_vec += einsum('jin,jn->ji', u, v); [sparsify at t==iters-2]
Identity used: b_vec(t) = einsum('jin,n->ji', u, W_t), W_t = sum_{tau<t} v_tau,
and avg_i b_vec = (Usum . W)/I with Usum = sum_i u (from iteration 0's matmul).

Per core (8 cores, batch-parallel): B=32, u [128 ip, (b, j, ic, n)] bf16 SBUF-
resident; i = ip*9 + ic so HBM runs are 576B. s-einsum on PE (cross-term
matmuls, diagonal extracted via ACT psum copy + diagonal-AP DMA); n-einsum on
DVE (bf16 2x mult + binary tree over n).
"""

import numpy as np

J = 10
I = 1152
N = 16
BATCH = 256
NCORES = 8
B = BATCH // NCORES
IP = 128
IC = I // IP                  # 9; i = ip*IC + ic
EPS = 1e-8
SPARSE_THRESHOLD = 0.1

GROUPS = [(0, 12), (12, 24), (24, 32)]
BC1 = 2                       # b per load chunk (16 chunks)
NCHUNK = B // BC1
FU = J * IC * N               # 1440 free elems per b in u16
FC = IC * J                   # 90 free elems per b in bt/e/c

_CACHE = {}


def _build(iters: int):
    assert iters != 2, 'iters==2 unsupported by the pipelined sparsify'
    import concourse.bass as bass
    from concourse import mybir
    import bass_rust

    f32 = mybir.dt.float32
    bf16 = mybir.dt.bfloat16
    AF = mybir.ActivationFunctionType
    ALU = mybir.AluOpType
    X = mybir.AxisListType.X

    def view(ap, dims, off=0):
        a = ap.copy()
        a.ap = bass_rust.VecI64Pair(dims)
        if off:
            a.offset = ap.offset + off
        return a

    # sub-chunk (4b) ranges of each drain group, as global kc indices
    KCS = [(0, 3), (3, 6), (6, 8)]       # kc covers b [4*kc, 4*kc+4)
    LASTKC = [3, 6, 8]                   # sNC threshold per group (count)
    CASTREQ = [6, 12, 16]                # cast chunks needed per group (t=0)
    MM0REQ = [6, 12, 16]                 # iter-0 MM chunks needed per group
    NITER_W = iters - 1                  # iterations that write W

    nc = bass.Bass("TRN2", target_bir_lowering=False, debug=False)

    u_d = nc.dram_tensor("u_hat", [B, J, I, N], f32, kind="ExternalInput")
    bias_d = nc.dram_tensor("bias", [J, N], f32, kind="ExternalInput")
    v_d = nc.dram_tensor("v", [B, J, N], f32, kind="ExternalOutput")
    wtmp_d = nc.dram_tensor("wtmp", [B * J * N], bf16)
    mtmp_d = nc.dram_tensor("mtmp", [B * J], bf16)

    import contextlib
    es = contextlib.ExitStack()
    with es:
        u16 = es.enter_context(nc.sbuf_tensor([128, B * FU], bf16))
        st0 = es.enter_context(nc.sbuf_tensor([128, BC1 * FU], f32))
        st1 = es.enter_context(nc.sbuf_tensor([128, BC1 * FU], f32))
        ones = es.enter_context(nc.sbuf_tensor([128, J], f32))
        t0 = es.enter_context(nc.sbuf_tensor([128, 4 * FU], bf16))
        t1 = es.enter_context(nc.sbuf_tensor([128, 4 * FU // 2], bf16))
        t2 = es.enter_context(nc.sbuf_tensor([128, 4 * FU // 4], bf16))
        t3 = es.enter_context(nc.sbuf_tensor([128, 4 * FU // 8], bf16))
        bt = es.enter_context(nc.sbuf_tensor([128, B * FC], f32))
        e16 = es.enter_context(nc.sbuf_tensor([128, B * FC], bf16))
        zsum = es.enter_context(nc.sbuf_tensor([128, B * IC], f32))
        rz = es.enter_context(nc.sbuf_tensor([128, B * IC], bf16))
        c16 = es.enter_context(nc.sbuf_tensor([128, B * FC], bf16))
        wrep = es.enter_context(nc.sbuf_tensor([128, B * J * N], bf16))
        mrep = es.enter_context(nc.sbuf_tensor([128, B * J], bf16))
        bias_sb = es.enter_context(nc.sbuf_tensor([J, N], f32))
        usum = es.enter_context(nc.sbuf_tensor([J, B * N], f32))
        wacc = es.enter_context(nc.sbuf_tensor([J, B * N], f32))
        w16 = es.enter_context(nc.sbuf_tensor([J, B * N], bf16))
        avgjb = es.enter_context(nc.sbuf_tensor([32, 32], f32))
        avgT = es.enter_context(nc.sbuf_tensor([32, 32], f32))
        m01T = es.enter_context(nc.sbuf_tensor([32, 32], f32))
        m01T16 = es.enter_context(nc.sbuf_tensor([32, J], bf16))
        m_jb = es.enter_context(nc.sbuf_tensor([32, 32], f32))
        ee = es.enter_context(nc.sbuf_tensor([32, J], f32))
        z2 = es.enter_context(nc.sbuf_tensor([32, 1], f32))
        lnz = es.enter_context(nc.sbuf_tensor([32, 1], f32))
        eps_b = es.enter_context(nc.sbuf_tensor([J, 1], f32))
        stil = es.enter_context(nc.sbuf_tensor([J, 2 * 1920], f32))
        sdt = es.enter_context(nc.sbuf_tensor([J, 3 * 192], f32))
        svt = es.enter_context(nc.sbuf_tensor([J, 3 * 192], f32))
        s2t = es.enter_context(nc.sbuf_tensor([J, 3 * 192], f32))
        sqt = es.enter_context(nc.sbuf_tensor([J, 3 * 12], f32))
        lnxt = es.enter_context(nc.sbuf_tensor([J, 3 * 12], f32))
        rst = es.enter_context(nc.sbuf_tensor([J, 3 * 12], f32))
        dent = es.enter_context(nc.sbuf_tensor([J, 3 * 12], f32))
        rect = es.enter_context(nc.sbuf_tensor([J, 3 * 12], f32))
        ggt = es.enter_context(nc.sbuf_tensor([J, 3 * 12], f32))
        vtt = es.enter_context(nc.sbuf_tensor([J, 3 * 192], f32))
        psA = es.enter_context(nc.psum_tensor([J, 2048], f32))
        psB = es.enter_context(nc.psum_tensor([J, 2048], f32))
        sLD = es.enter_context(nc.semaphore())
        sLDH = es.enter_context(nc.semaphore())
        sCAST = es.enter_context(nc.semaphore())
        sNC = es.enter_context(nc.semaphore())
        sEXP = es.enter_context(nc.semaphore())
        sC = es.enter_context(nc.semaphore())
        sMM = es.enter_context(nc.semaphore())
        sMM0 = es.enter_context(nc.semaphore())
        sSTIL = es.enter_context(nc.semaphore())
        sDIAG = es.enter_context(nc.semaphore())
        sSQ1 = es.enter_context(nc.semaphore())
        sSQ2 = es.enter_context(nc.semaphore())
        sVW = es.enter_context(nc.semaphore())
        sW16 = es.enter_context(nc.semaphore())
        sWT = es.enter_context(nc.semaphore())
        sWREP = es.enter_context(nc.semaphore())
        sSPD = es.enter_context(nc.semaphore())
        sSPA = es.enter_context(nc.semaphore())
        sMT = es.enter_context(nc.semaphore())
        sMREP = es.enter_context(nc.semaphore())
        sBIAS = es.enter_context(nc.semaphore())
        sINIT = es.enter_context(nc.semaphore())
        sVOUT = es.enter_context(nc.semaphore())
        block = es.enter_context(nc.Block())
        PS = [psA, psB]

        def u_chunk_src(k):
            return view(
                u_d[:],
                [[IC * N, 128], [J * I * N, BC1], [I * N, J], [1, IC * N]],
                off=k * BC1 * J * I * N,
            )

        def u_chunk_dst(stg):
            return view(
                stg[:], [[BC1 * FU, 128], [FU, BC1], [IC * N, J], [1, IC * N]])

        def load_chunk(eng, k, sem):
            eng.dma_start(u_chunk_dst((st0, st1)[k % 2]),
                          u_chunk_src(k)).then_inc(sem, 16)

        # -------------------- PL: first two loads --------------------
        @block.gpsimd
        def _(pl):
            load_chunk(pl, 0, sLD)
            pl.wait_ge(sLD, 16)
            load_chunk(pl, 1, sLD)

        # -------------------- ACT --------------------
        @block.scalar
        def _(act):
            def stil_emit(t, g):
                gi = t * 3 + g
                gb = GROUPS[g][1] - GROUPS[g][0]
                nbank = ((gb + 2) // 3)
                if t == 0:
                    act.wait_ge(sMM0, MM0REQ[g])
                else:
                    act.wait_ge(sMM, (t - 1) * 3 + g + 1)
                if gi >= 2:
                    act.wait_ge(sDIAG, 16 * (gi - 1))
                nc.scalar.copy(
                    view(stil[:], [[2 * 1920, 10], [480, nbank], [1, 480]],
                         off=(gi % 2) * 1920),
                    view(PS[gi % 2][:], [[2048, 10], [512, nbank], [1, 480]]),
                ).then_inc(sSTIL, 1)

            def lnrs_emit(t, g):
                gi = t * 3 + g
                gb = GROUPS[g][1] - GROUPS[g][0]
                act.wait_ge(sSQ1, gi + 1)
                nc.scalar.activation(
                    view(lnxt[:], [[36, 10], [1, gb]], off=g * 12),
                    view(sqt[:], [[36, 10], [1, gb]], off=g * 12),
                    AF.Ln, bias=eps_b[:],
                )
                nc.scalar.drain()
                nc.scalar.activation(
                    view(rst[:], [[36, 10], [1, gb]], off=g * 12),
                    view(lnxt[:], [[36, 10], [1, gb]], off=g * 12),
                    AF.Exp, scale=-0.5,
                ).then_inc(sSQ2, 1)

            def w16_emit(t, g):
                gi = t * 3 + g
                g0, g1 = GROUPS[g]
                gb = g1 - g0
                act.wait_ge(sVW, gi + 1)
                if t >= 1:
                    act.wait_ge(sWT, 16 * ((t - 1) * 3 + g + 1))
                nc.scalar.copy(
                    view(w16[:], [[B * N, 10], [1, gb * N]], off=g0 * N),
                    view(wacc[:], [[B * N, 10], [1, gb * N]], off=g0 * N),
                ).then_inc(sW16, 1)

            def exp_emit(t, g):
                act.wait_ge(sNC, (t - 1) * 8 + LASTKC[g])
                if t >= 2:
                    act.wait_ge(sC, (t - 2) * 3 + g + 1)
                g0, g1 = GROUPS[g]
                gb = g1 - g0
                nc.scalar.activation(
                    view(e16[:], [[B * FC, 128], [1, gb * FC]], off=g0 * FC),
                    view(bt[:], [[B * FC, 128], [1, gb * FC]], off=g0 * FC),
                    AF.Exp,
                ).then_inc(sEXP, 1)

            def sparsify_act_emit(g):
                g0, g1 = GROUPS[g]
                gb = g1 - g0
                act.wait_ge(sSPD, 3 * g + 1)
                nc.scalar.activation(
                    ee[:], view(avgT[:], [[32, 32], [1, J]]), AF.Exp,
                ).then_inc(sSPA, 1)
                act.wait_ge(sSPD, 3 * g + 2)
                nc.scalar.activation(lnz[:], z2[:], AF.Ln).then_inc(sSPA, 1)
                act.wait_ge(sSPD, 3 * g + 3)
                nc.scalar.copy(
                    m01T16[:], view(m01T[:], [[32, 32], [1, J]]),
                ).then_inc(sSPA, 1)

            act.wait_ge(sINIT, 4)
            for k in range(NCHUNK):
                if k < 2:
                    act.wait_ge(sLD, 16 * (k + 1))
                else:
                    act.wait_ge(sLDH, 16 * (k - 1))
                nc.scalar.activation(
                    view(u16[:], [[B * FU, 128], [1, BC1 * FU]],
                         off=k * BC1 * FU),
                    (st0, st1)[k % 2][:],
                    AF.Copy,
                ).then_inc(sCAST, 1)
                if k + 2 < NCHUNK:
                    act.wait_ge(sCAST, k + 1)
                    act.wait_ge(sMM0, k + 1)
                    load_chunk(act, k + 2, sLDH)
                if k == 5:
                    stil_emit(0, 0)
                if k == 6:
                    lnrs_emit(0, 0)
                    if iters > 1:
                        w16_emit(0, 0)
                if k == 11:
                    stil_emit(0, 1)
                    if iters > 1:
                        exp_emit(1, 0)
                if k == 12:
                    lnrs_emit(0, 1)
                    if iters > 1:
                        w16_emit(0, 1)
                if k == 13 and iters > 1:
                    exp_emit(1, 1)

            stil_emit(0, 2)
            lnrs_emit(0, 2)
            if iters > 1:
                w16_emit(0, 2)
                exp_emit(1, 2)
                for t in range(1, iters):
                    last_t = t == iters - 1
                    for g in range(3):
                        stil_emit(t, g)
                        lnrs_emit(t, g)
                        if not last_t:
                            w16_emit(t, g)
                        if t == iters - 2:
                            sparsify_act_emit(g)
                        if t + 1 <= iters - 1:
                            exp_emit(t + 1, g)

        # -------------------- DVE --------------------
        class _VD:
            def __getattr__(self, name):
                fn = getattr(nc.vector, name)

                def wrapped(*a, **k):
                    r = fn(*a, **k)
                    nc.vector.drain()
                    return r
                return wrapped

        vd = _VD()

        @block.vector
        def _(dve):
            def nc_emit(t, g):
                g0, g1 = GROUPS[g]
                dve.wait_ge(sWREP, 16 * ((t - 1) * 3 + g + 1))
                if t == 1:
                    dve.wait_ge(sCAST, CASTREQ[g])
                if t >= 2:
                    dve.wait_ge(sEXP, (t - 2) * 3 + g + 1)
                for kc in range(*KCS[g]):
                    ob = kc * 4
                    vd.tensor_mul(
                        view(t0[:], [[4 * FU, 128], [IC * N, 4 * J],
                                     [N, IC], [1, N]]),
                        view(u16[:], [[B * FU, 128], [IC * N, 4 * J],
                                     [N, IC], [1, N]], off=ob * FU),
                        view(wrep[:], [[B * J * N, 128], [N, 4 * J],
                                     [0, IC], [1, N]], off=ob * J * N),
                    )
                    for (src_t, dst_t, w) in ((t0, t1, 8), (t1, t2, 4),
                                              (t2, t3, 2)):
                        si = [[4 * J * IC * 2 * w, 128],
                              [2 * w, 4 * J * IC], [1, w]]
                        vd.tensor_add(
                            view(dst_t[:], [[4 * J * IC * w, 128],
                                            [w, 4 * J * IC], [1, w]]),
                            view(src_t[:], si),
                            view(src_t[:], si, off=w),
                        )
                    vd.tensor_add(
                        view(bt[:], [[B * FC, 128], [FC, 4], [1, J],
                                     [J, IC]], off=ob * FC),
                        view(t3[:], [[4 * J * IC * 2, 128],
                                     [J * IC * 2, 4], [2 * IC, J],
                                     [2, IC]]),
                        view(t3[:], [[4 * J * IC * 2, 128],
                                     [J * IC * 2, 4], [2 * IC, J],
                                     [2, IC]], off=1),
                    ).then_inc(sNC, 1)

            def sm_emit(t, g):
                g0_, g1_ = GROUPS[g]
                gb_ = g1_ - g0_
                last = t == iters - 1
                dve.wait_ge(sEXP, (t - 1) * 3 + g + 1)
                if last and iters > 1:
                    dve.wait_ge(sMREP, 16 * (g + 1))
                    vd.tensor_mul(
                        view(e16[:], [[B * FC, 128], [FC, gb_],
                                     [J, IC], [1, J]], off=g0_ * FC),
                        view(e16[:], [[B * FC, 128], [FC, gb_],
                                     [J, IC], [1, J]], off=g0_ * FC),
                        view(mrep[:], [[B * J, 128], [J, gb_],
                                     [0, IC], [1, J]], off=g0_ * J),
                    )
                vd.reduce_sum(
                    view(zsum[:], [[B * IC, 128], [1, gb_ * IC]],
                         off=g0_ * IC),
                    view(e16[:], [[B * FC, 128], [J, gb_ * IC], [1, J]],
                         off=g0_ * FC),
                    axis=X,
                )
                with nc.allow_low_precision(reason="softmax recip bf16"):
                    vd.reciprocal(
                        view(rz[:], [[B * IC, 128], [1, gb_ * IC]],
                             off=g0_ * IC),
                        view(zsum[:], [[B * IC, 128], [1, gb_ * IC]],
                             off=g0_ * IC),
                    )
                if t >= 2:
                    dve.wait_ge(sMM, (t - 2) * 3 + g + 1)
                vd.tensor_mul(
                    view(c16[:], [[B * FC, 128], [J, gb_ * IC], [1, J]],
                         off=g0_ * FC),
                    view(e16[:], [[B * FC, 128], [J, gb_ * IC], [1, J]],
                         off=g0_ * FC),
                    view(rz[:], [[B * IC, 128], [1, gb_ * IC], [0, J]],
                         off=g0_ * IC),
                ).then_inc(sC, 1)

            def sq_emit(t, g):
                gi = t * 3 + g
                g0, g1 = GROUPS[g]
                gb = g1 - g0
                o12 = g * 12
                o192 = g * 192
                last = t == iters - 1
                dve.wait_ge(sDIAG, 16 * (gi + 1))
                if t == 0 and g == 0:
                    dve.wait_ge(sBIAS, 16)
                if t == 0:
                    vd.tensor_copy(
                        view(usum[:], [[B * N, 10], [1, gb * N]], off=g0 * N),
                        view(sdt[:], [[3 * 192, 10], [1, gb * N]], off=o192),
                    )
                vd.scalar_tensor_tensor(
                    view(svt[:], [[3 * 192, 10], [N, gb], [1, N]], off=o192),
                    view(sdt[:], [[3 * 192, 10], [N, gb], [1, N]], off=o192),
                    (1.0 / J) if t == 0 else 1.0,
                    view(bias_sb[:], [[N, 10], [0, gb], [1, N]]),
                    ALU.mult, ALU.add,
                )
                vd.tensor_mul(
                    view(s2t[:], [[3 * 192, 10], [1, gb * N]], off=o192),
                    view(svt[:], [[3 * 192, 10], [1, gb * N]], off=o192),
                    view(svt[:], [[3 * 192, 10], [1, gb * N]], off=o192),
                )
                vd.reduce_sum(
                    view(sqt[:], [[36, 10], [1, gb]], off=o12),
                    view(s2t[:], [[3 * 192, 10], [N, gb], [1, N]], off=o192),
                    axis=X,
                ).then_inc(sSQ1, 1)
                dve.wait_ge(sSQ2, gi + 1)
                vd.tensor_scalar_add(
                    view(dent[:], [[36, 10], [1, gb]], off=o12),
                    view(sqt[:], [[36, 10], [1, gb]], off=o12), 1.0)
                vd.reciprocal(
                    view(rect[:], [[36, 10], [1, gb]], off=o12),
                    view(dent[:], [[36, 10], [1, gb]], off=o12))
                vd.tensor_mul(
                    view(ggt[:], [[36, 10], [1, gb]], off=o12),
                    view(sqt[:], [[36, 10], [1, gb]], off=o12),
                    view(rect[:], [[36, 10], [1, gb]], off=o12))
                vd.tensor_mul(
                    view(ggt[:], [[36, 10], [1, gb]], off=o12),
                    view(ggt[:], [[36, 10], [1, gb]], off=o12),
                    view(rst[:], [[36, 10], [1, gb]], off=o12))
                vd.tensor_mul(
                    view(vtt[:], [[3 * 192, 10], [N, gb], [1, N]], off=o192),
                    view(svt[:], [[3 * 192, 10], [N, gb], [1, N]], off=o192),
                    view(ggt[:], [[36, 10], [1, gb], [0, N]], off=o12))
                if last:
                    if iters > 1:
                        vd.tensor_mul(
                            view(vtt[:], [[3 * 192, 10], [N, gb], [1, N]],
                                 off=o192),
                            view(vtt[:], [[3 * 192, 10], [N, gb], [1, N]],
                                 off=o192),
                            view(m_jb[:], [[32, 10], [1, gb], [0, N]],
                                 off=g0))
                    vd.engine_nop().then_inc(sVW, 1)
                else:
                    wv = view(wacc[:], [[B * N, 10], [1, gb * N]], off=g0 * N)
                    if t >= 1:
                        dve.wait_ge(sW16, (t - 1) * 3 + g + 1)
                    if t == 0:
                        vd.tensor_copy(
                            wv, view(vtt[:], [[3 * 192, 10], [1, gb * N]],
                                     off=o192)).then_inc(sVW, 1)
                    else:
                        vd.tensor_add(
                            wv, wv, view(vtt[:], [[3 * 192, 10],
                                         [1, gb * N]], off=o192)
                        ).then_inc(sVW, 1)

            def sparsify_emit(g):
                g0, g1 = GROUPS[g]
                gb = g1 - g0
                vd.tensor_mul(
                    view(s2t[:], [[3 * 192, 10], [1, gb * N]], off=g0 * N),
                    view(usum[:], [[B * N, 10], [1, gb * N]], off=g0 * N),
                    view(wacc[:], [[B * N, 10], [1, gb * N]], off=g0 * N))
                vd.reduce_sum(
                    view(avgjb[:], [[32, 10], [1, gb]], off=g0),
                    view(s2t[:], [[3 * 192, 10], [N, gb], [1, N]], off=g0 * N),
                    axis=X,
                )
                vd.tensor_scalar_mul(
                    view(avgjb[:], [[32, 10], [1, gb]], off=g0),
                    view(avgjb[:], [[32, 10], [1, gb]], off=g0),
                    1.0 / I,
                )
                vd.transpose(avgT[:], avgjb[:]).then_inc(sSPD, 1)
                dve.wait_ge(sSPA, 3 * g + 1)
                vd.reduce_sum(z2[:], ee[:], axis=X).then_inc(sSPD, 1)
                dve.wait_ge(sSPA, 3 * g + 2)
                vd.tensor_scalar(
                    view(m01T[:], [[32, 32], [1, J]]),
                    view(avgT[:], [[32, 32], [1, J]]),
                    lnz[:],
                    float(np.log(SPARSE_THRESHOLD)),
                    ALU.subtract,
                    ALU.is_ge,
                )
                vd.transpose(m_jb[:], m01T[:]).then_inc(sSPD, 1)

            vd.memset(ones[:], 1.0).then_inc(sINIT, 1)
            vd.memset(avgjb[:], 0.0).then_inc(sINIT, 1)
            vd.memset(m01T[:], 0.0).then_inc(sINIT, 1)
            vd.memset(eps_b[:], EPS).then_inc(sINIT, 1)

            if iters == 1:
                for g in range(3):
                    sq_emit(0, g)
            else:
                for t in range(1, iters):
                    last_t = t == iters - 1
                    for g in range(3):
                        sq_emit(t - 1, g)
                        if last_t and iters > 1:
                            sparsify_emit(g)
                        nc_emit(t, g)
                        sm_emit(t, g)
                    if last_t:
                        for g in range(3):
                            sq_emit(t, g)

        # -------------------- PE --------------------
        @block.tensor
        def _(pe):
            pe.wait_ge(sINIT, 4)
            # iteration 0 in fp32, streamed from the staging chunks during load
            grp_of = lambda b: 0 if b < 12 else (1 if b < 24 else 2)
            for k in range(NCHUNK):
                if k < 2:
                    pe.wait_ge(sLD, 16 * (k + 1))
                else:
                    pe.wait_ge(sLDH, 16 * (k - 1))
                stg = (st0, st1)[k % 2]
                mm = None
                for bl in range(BC1):
                    b = k * BC1 + bl
                    g = grp_of(b)
                    g0 = GROUPS[g][0]
                    if b == 24:
                        pe.wait_ge(sSTIL, 1)     # ps slot 0 reuse (g2)
                    q = b - g0
                    off = (q // 3) * 512 + (q % 3) * 160
                    ps = PS[g % 2]
                    for ic in range(IC):
                        mm = nc.tensor.matmul(
                            view(ps[:], [[2048, 10], [1, 160]], off=off),
                            ones[:],
                            view(stg[:], [[BC1 * FU, 128], [IC * N, J],
                                         [1, N]], off=bl * FU + ic * N),
                            start=(ic == 0), stop=(ic == IC - 1),
                        )
                if k == NCHUNK - 1:
                    # fill the unused 9th slot of group 2 (dup of b=31)
                    for ic in range(IC):
                        mm = nc.tensor.matmul(
                            view(PS[0][:], [[2048, 10], [1, 160]],
                                 off=2 * 512 + 2 * 160),
                            ones[:],
                            view(stg[:], [[BC1 * FU, 128], [IC * N, J],
                                         [1, N]], off=(BC1 - 1) * FU + ic * N),
                            start=(ic == 0), stop=(ic == IC - 1),
                        )
                mm.then_inc(sMM0, 1)
            for t in range(1, iters):
                for g in range(3):
                    gi = t * 3 + g
                    g0, g1 = GROUPS[g]
                    gb = g1 - g0
                    nslot = ((gb + 2) // 3) * 3
                    pe.wait_ge(sC, (t - 1) * 3 + g + 1)
                    pe.wait_ge(sSTIL, gi - 1)
                    ps = PS[gi % 2]
                    mm = None
                    for q in range(nslot):
                        b = min(g0 + q, B - 1)
                        off = (q // 3) * 512 + (q % 3) * 160
                        for ic in range(IC):
                            lh = view(c16[:], [[B * FC, 128], [1, J]],
                                      off=b * FC + ic * J)
                            mm = nc.tensor.matmul(
                                view(ps[:], [[2048, 10], [1, 160]], off=off),
                                lh,
                                view(u16[:], [[B * FU, 128], [IC * N, J],
                                             [1, N]], off=b * FU + ic * N),
                                start=(ic == 0), stop=(ic == IC - 1),
                            )
                    mm.then_inc(sMM, 1)

        # -------------------- SP --------------------
        @block.sync
        def _(sp):
            sp.dma_start(bias_sb[:], bias_d[:]).then_inc(sBIAS, 16)
            for t in range(iters):
                last = t == iters - 1
                for g in range(3):
                    gi = t * 3 + g
                    g0, g1 = GROUPS[g]
                    gb = g1 - g0
                    nslot = ((gb + 2) // 3) * 3
                    nbank = nslot // 3
                    sp.wait_ge(sSTIL, gi + 1)
                    if t >= 1:
                        sp.wait_ge(sVW, (t - 1) * 3 + g + 1)
                    sp.wait_ge(sDIAG, 16 * gi)
                    sp.dma_start(
                        view(sdt[:], [[3 * 192, 10], [N * 3, nbank], [N, 3],
                                     [1, N]], off=g * 192),
                        view(stil[:], [[2 * 1920 + N, 10], [480, nbank],
                                     [160, 3], [1, N]], off=(gi % 2) * 1920),
                    ).then_inc(sDIAG, 16)
                    if not last:
                        wg = t * 3 + g
                        sp.wait_ge(sW16, gi + 1)
                        sp.dma_start(
                            view(wtmp_d[:], [[N, J], [J * N, gb], [1, N]],
                                 off=g0 * J * N),
                            view(w16[:], [[B * N, 10], [N, gb], [1, N]],
                                 off=g0 * N),
                        ).then_inc(sWT, 16)
                        sp.wait_ge(sWT, 16 * (wg + 1))
                        if t >= 1:
                            # wrep slice rewrite guard: DVE mults of iter t done
                            sp.wait_ge(sNC, (t - 1) * 8 + LASTKC[g])
                        sp.wait_ge(sWREP, 16 * wg)
                        sp.dma_start(
                            view(wrep[:], [[B * J * N, 128], [1, gb * J * N]],
                                 off=g0 * J * N),
                            view(wtmp_d[:], [[0, 128], [1, gb * J * N]],
                                 off=g0 * J * N),
                        ).then_inc(sWREP, 16)
                        if t == iters - 2:
                            sp.wait_ge(sSPA, 3 * g + 3)
                            sp.wait_ge(sMT, 16 * g)
                            sp.dma_start(
                                view(mtmp_d[:], [[J, gb], [1, J]],
                                     off=g0 * J),
                                view(m01T16[:], [[J, gb], [1, J]],
                                     off=g0 * J),
                            ).then_inc(sMT, 16)
                            sp.wait_ge(sMT, 16 * (g + 1))
                            sp.wait_ge(sMREP, 16 * g)
                            sp.dma_start(
                                view(mrep[:], [[B * J, 128], [1, gb * J]],
                                     off=g0 * J),
                                view(mtmp_d[:], [[0, 128], [1, gb * J]],
                                     off=g0 * J),
                            ).then_inc(sMREP, 16)
                    else:
                        sp.wait_ge(sVW, gi + 1)
                        sp.wait_ge(sVOUT, 16 * g)
                        sp.dma_start(
                            view(v_d[:], [[N, 10], [J * N, gb], [1, N]],
                                 off=g0 * J * N),
                            view(vtt[:], [[3 * 192, 10], [N, gb], [1, N]],
                                 off=g * 192),
                        ).then_inc(sVOUT, 16)

            sp.wait_ge(sVOUT, 48)

    return nc


def _get_program(iters: int):
    if iters not in _CACHE:
        _CACHE[iters] = _build(iters)
    return _CACHE[iters]


def _enable_ldw_opt():
    """Dedupe back-to-back identical LDWEIGHTS (the iter-0 ones matrix)."""
    import concourse.bass_utils as BU
    if getattr(BU, "_ldw_patched", False):
        return
    _orig = BU.run_command

    def _patched(cmd, *a, **k):
        if isinstance(cmd, list):
            cmd = ["--enable-ldw-opt=true" if c == "--enable-ldw-opt=false"
                   else c for c in cmd]
        return _orig(cmd, *a, **k)

    BU.run_command = _patched
    BU._ldw_patched = True


def kernel(u_hat=None, bias=None, iters=3, **kw):
    from concourse.bass_utils import run_bass_kernel_spmd
    _enable_ldw_opt()

    iters = int(iters)
    u_hat = np.ascontiguousarray(np.asarray(u_hat, dtype=np.float32))
    bias = np.ascontiguousarray(np.asarray(bias, dtype=np.float32))
    assert u_hat.shape == (BATCH, J, I, N), u_hat.shape

    nc = _get_program(iters)
    in_maps = [
        {"u_hat": u_hat[c * B:(c + 1) * B], "bias": bias}
        for c in range(NCORES)
    ]
    res = run_bass_kernel_spmd(nc, in_maps, list(range(NCORES)))
    out = np.concatenate([res.results[c]["v"] for c in range(NCORES)], axis=0)
    return out.astype(np.float32)


if __name__ == "__main__":
    rng = np.random.default_rng(0)
    u = rng.standard_normal((BATCH, J, I, N), dtype=np.float32)
    b = np.full((J, N), 0.1, np.float32)
    v = kernel(u_hat=u, bias=b, iters=3)
    print(v.shape, v.dtype, np.abs(v).max())

